# revision 32
# baseline (speedup 1.0000x reference)
"""Trainium2 Bass kernel for nn_MultiLabelRNN.

Reference semantics (per batch row b, T=65 steps):
  embed0 = inputs @ W_embed.T + b_embed
  step: pi = embed @ W_in.T + b_in ; ps = state @ W_state.T + b_state
        gates (sigmoid/tanh), highway; logits = out @ W_out.T + b_out
        dist = softmax(logits); onehot = (dist == max(dist[:,1:]))
        next embed = onehot @ W_onehot.T + b_onehot
Key restructurings:
  * embed_{t+1} has only 151 possible values -> pi_{t+1} = onehotAug @ P_aug where
    P_aug = [W_onehot[:,c] @ W_in.T rows ; bias row], bias row folds
    b_onehot@W_in.T + b_in + [b_state|0].  Chunk-5 columns (the highway skip
    input pi5) are pre-halved so the DVE chain can use them directly.
  * All matmuls run as float32r (TF32-like, full PE rate at N>=256) in split
    form: x@W ~= xq@Wq + xq@Wr + xr@Wq with q/r an exact hi/lo decomposition
    (11-bit mantissa pieces; products are exact on the PE, fp32 accumulate in
    PSUM) -> fp32-grade accuracy at ~3x the speed of native fp32 matmul.
    Constant weights are split on the host; the recurrent state is split on
    device (ACT cast + DVE subtract).
  * Gates use tanh only (ACT tanh is ~4x more accurate than ACT sigmoid):
    sigmoid(x) = 0.5 + 0.5*tanh(x/2), memory is carried doubled (M2 = 2*mem).
  * argmax/onehot from logits (softmax is monotone); dists are computed in a
    decoupled end phase from logits stored to an HBM scratch buffer.
Sharding: data-parallel over batch, 128 rows per core on 8 cores.
"""

import numpy as np

import concourse.bacc as bacc
import concourse.mybir as mybir
from concourse.tile import TileContext
from concourse.bass_utils import run_bass_kernel_spmd

B, D, H, C, T = 1024, 2048, 512, 151, 65
NCORES = 8
BC = B // NCORES  # 128 batch rows per core
H6 = 6 * H
NTILE = H6 // 512  # 6 preact tiles of 512 columns
NOUT = 256  # padded logits width (f32r needs N>=256 for full rate)
CA = C + 1  # onehot rows + constant-one bias row

f32 = mybir.dt.float32
f32r = mybir.dt.float32r
i32 = mybir.dt.int32
u32 = mybir.dt.uint32
AF = mybir.ActivationFunctionType
ALU = mybir.AluOpType
AX = mybir.AxisListType

_CACHE = {}


def round12(x):
    """Round fp32 to 11 explicit mantissa bits (f32r grid), half-to-even.
    Matches the device's f32r cast bit-exactly (verified on HW)."""
    v = np.ascontiguousarray(x, dtype=np.float32).view(np.uint32)
    keep = v & np.uint32(0xFFFFF000)
    rem = v & np.uint32(0x00000FFF)
    lsb = (v >> np.uint32(12)) & np.uint32(1)
    up = (rem > 0x800) | ((rem == 0x800) & (lsb == 1))
    return (keep + (up.astype(np.uint32) << np.uint32(12))).view(np.float32)


def split12(x):
    q = round12(x)
    r = round12((x.astype(np.float32) - q).astype(np.float32))
    return q, r


def build_kernel(n_steps=T):
    nc = bacc.Bacc(target_bir_lowering=False)

    # ---- I/O ----
    x_d = nc.dram_tensor("x", [BC, D], f32, kind="ExternalInput")
    wsq_d = nc.dram_tensor("wsq", [H, 5 * H], f32r, kind="ExternalInput")
    wsr_d = nc.dram_tensor("wsr", [H, 5 * H], f32r, kind="ExternalInput")
    paq0_d = nc.dram_tensor("paq0", [128, H6], f32r, kind="ExternalInput")
    par0_d = nc.dram_tensor("par0", [128, H6], f32r, kind="ExternalInput")
    # stacked chunk-1 table: rows 0..23 = q piece, rows 32..55 = r piece,
    # zero padding rows 24..31 / 56..63 (lets one K=64 matmul do both terms)
    pa1s_d = nc.dram_tensor("pa1s", [64, H6], f32r, kind="ExternalInput")
    woq_d = nc.dram_tensor("woq", [H, NOUT], f32r, kind="ExternalInput")
    wor_d = nc.dram_tensor("wor", [H, NOUT], f32r, kind="ExternalInput")
    bo2_d = nc.dram_tensor("bo2", [2, NOUT], f32r, kind="ExternalInput")
    wembT_d = nc.dram_tensor("wembT", [D, H], f32, kind="ExternalInput")
    bemb_d = nc.dram_tensor("bemb", [1, H], f32, kind="ExternalInput")
    winT_d = nc.dram_tensor("winT", [H, H6], f32, kind="ExternalInput")
    binrow_d = nc.dram_tensor("binrow", [1, H6], f32, kind="ExternalInput")
    mask_d = nc.dram_tensor("maskm", [BC, T], f32, kind="ExternalInput")
    ident_d = nc.dram_tensor("ident", [128, 128], f32, kind="ExternalInput")
    onesr_d = nc.dram_tensor("onesr", [1, 128], f32, kind="ExternalInput")
    ones2_d = nc.dram_tensor("ones2", [2, 128], f32r, kind="ExternalInput")

    dists_o = nc.dram_tensor("dists_o", [BC, T * C], f32, kind="ExternalOutput")
    best_o = nc.dram_tensor("best_o", [BC, T], i32, kind="ExternalOutput")
    states_o = nc.dram_tensor("states_o", [BC, T, H], f32, kind="ExternalOutput")

    with TileContext(nc) as tc:
        with (
            tc.tile_pool(name="tiny", bufs=1) as tiny,
            tc.tile_pool(name="pre_ps", bufs=6, space="PSUM") as pre_ps,
            tc.tile_pool(name="tr_ps", bufs=2, space="PSUM") as tr_ps,
        ):
            ident = tiny.tile([128, 128], f32)
            onesr = tiny.tile([1, 128], f32)
            onesr_r = tiny.tile([1, 128], f32r)
            nc.sync.dma_start(ident, ident_d[:, :])
            nc.sync.dma_start(onesr, onesr_d[:, :])
            nc.vector.tensor_copy(onesr_r, onesr)

            def pre_tile():
                t_ = pre_ps.tile([128, 512], f32, tag="pre", name="pre_t")
                return t_

            # ================= init phase: embed0 -> preact_0 =================
            pre_tiles = None
            with tc.tile_pool(name="init", bufs=1) as initp:
                x_bm = initp.tile([128, D], f32)
                nc.sync.dma_start(x_bm, x_d[:, :])
                # transpose inputs -> xT (D x 128) as 16 chunks
                xT = initp.tile([128, D // 128, 128], f32)
                for k in range(D // 128):
                    xt_ps = tr_ps.tile([128, 512], f32, tag="tr", name="xt_ps")
                    nc.tensor.transpose(xt_ps[:, 0:128], x_bm[:, k * 128:(k + 1) * 128], ident)
                    nc.scalar.copy(xT[:, k, :], xt_ps[:, 0:128])
                # embed0 (batch-major) = x @ W_embed.T + b_embed  (fp32 matmuls)
                bemb_sb = initp.tile([1, H], f32)
                nc.sync.dma_start(bemb_sb, bemb_d[:, :])
                e0_ps = tr_ps.tile([128, 512], f32, tag="tr", name="e0_ps")
                with tc.tile_pool(name="wemb", bufs=2) as wembp:
                    for k in range(D // 128):
                        wch = wembp.tile([128, H], f32, tag="wch", name="wch")
                        nc.sync.dma_start(wch, wembT_d[k * 128:(k + 1) * 128, :])
                        nc.tensor.matmul(e0_ps, xT[:, k, :], wch, start=(k == 0), stop=False)
                    nc.tensor.matmul(e0_ps, onesr, bemb_sb, start=False, stop=True)
                e0_bm = initp.tile([128, H], f32)
                nc.scalar.copy(e0_bm, e0_ps)
                # transpose embed0 -> e0T (4 chunks)
                e0T_ps = tr_ps.tile([128, 512], f32, tag="tr", name="e0T_ps")
                for k in range(4):
                    nc.tensor.transpose(e0T_ps[:, k * 128:(k + 1) * 128],
                                        e0_bm[:, k * 128:(k + 1) * 128], ident)
                e0T = initp.tile([128, 4, 128], f32)
                nc.scalar.copy(e0T, e0T_ps.rearrange("p (k m) -> p k m", k=4))
                # preact_0 = embed0 @ W_in.T + (b_in + b_state_pad)   (fp32)
                binrow_sb = initp.tile([1, H6], f32)
                nc.sync.dma_start(binrow_sb, binrow_d[:, :])
                pre_tiles = [pre_tile() for _ in range(NTILE)]
                with tc.tile_pool(name="winp", bufs=2) as winp:
                    for k in range(4):
                        wik = winp.tile([128, H6], f32, tag="wik", name="wik")
                        nc.sync.dma_start(wik, winT_d[k * 128:(k + 1) * 128, :])
                        for j in range(NTILE):
                            nc.tensor.matmul(pre_tiles[j], e0T[:, k, :],
                                             wik[:, j * 512:(j + 1) * 512],
                                             start=(k == 0), stop=False)
                    for j in range(NTILE):
                        nc.tensor.matmul(pre_tiles[j], onesr,
                                         binrow_sb[:, j * 512:(j + 1) * 512],
                                         start=False, stop=True)

            # ================= constants for the main loop =================
            with (
                tc.tile_pool(name="const", bufs=1) as constp,
                tc.tile_pool(name="work", bufs=1) as workp,
                tc.tile_pool(name="work2", bufs=2) as workp2,
            ):
                wsq = constp.tile([128, 4, 5 * H], f32r)
                wsr = constp.tile([128, 4, 5 * H], f32r)
                nc.sync.dma_start(wsq, wsq_d.rearrange("(k p) n -> p k n", p=128))
                nc.sync.dma_start(wsr, wsr_d.rearrange("(k p) n -> p k n", p=128))
                paq0 = constp.tile([128, H6], f32r)
                par0 = constp.tile([128, H6], f32r)
                pa1s = constp.tile([64, H6], f32r)
                nc.sync.dma_start(paq0, paq0_d[:, :])
                nc.sync.dma_start(par0, par0_d[:, :])
                nc.sync.dma_start(pa1s, pa1s_d[:, :])
                woq = constp.tile([128, 4, NOUT], f32r)
                wor = constp.tile([128, 4, NOUT], f32r)
                nc.sync.dma_start(woq, woq_d.rearrange("(k p) n -> p k n", p=128))
                nc.sync.dma_start(wor, wor_d.rearrange("(k p) n -> p k n", p=128))
                bo2 = constp.tile([2, NOUT], f32r)
                nc.sync.dma_start(bo2, bo2_d[:, :])
                ones2_r = constp.tile([2, 128], f32r)
                nc.sync.dma_start(ones2_r, ones2_d[:, :])
                maskm = constp.tile([BC, T], f32)
                nc.sync.dma_start(maskm, mask_d[:, :])
                best_buf = constp.tile([BC, T], i32)

                # onehot batch-major staging: cols 0..150 = compare output,
                # col 151 = constant 1.0 (bias-row selector).  Persistent
                # tensors (not pool-rotated) so the constant column survives.
                oh_bm = constp.tile([128, 128], f32)
                # oh2: cols 0..22 = onehot classes 128..150 (q side), col 23 = 1
                # (bias-row selector), cols 32..54 = classes again (r side),
                # col 55 = 1, rest zero.  Its transpose is the stacked chunk-1
                # lhsT for the single K=64 pi matmul.
                oh2 = constp.tile([128, 64], f32)
                nc.vector.memset(oh2, 0.0)
                nc.vector.memset(oh2[:, 23:24], 1.0)
                nc.vector.memset(oh2[:, 55:56], 1.0)
                ohT0 = constp.tile([128, 128], f32r)
                ohT1 = constp.tile([64, 128], f32r)

                # M2 (= 2*memory) ping-pong, init zero
                m2 = workp2.tile([128, H], f32, tag="m2", name="m2")
                nc.vector.memset(m2, 0.0)

                CH = 5  # inline-softmax chunk (steps)
                lg_hist = []  # (t, lg_sb tile) pending softmax

                def softmax_chunk():
                    """Emit softmax for the pending lg tiles (overlaps PE work)."""
                    cn = len(lg_hist)
                    if cn == 0:
                        return
                    c0 = lg_hist[0][0]
                    ex = workp2.tile([128, CH * C], f32, tag="ex", name="ex")
                    for s, (tt, lgt) in enumerate(lg_hist):
                        nmx = workp.tile([128, 1], f32, tag="nmx", name="nmx")
                        nc.vector.tensor_reduce(nmx, lgt, axis=AX.X, op=ALU.max,
                                                negate=True)
                        nc.scalar.activation(ex[:, s * C:(s + 1) * C], lgt, AF.Exp,
                                             bias=nmx)
                    sm = workp.tile([128, CH], f32, tag="sm", name="sm")
                    nc.vector.tensor_reduce(sm[:, 0:cn],
                                            ex[:, 0:cn * C].rearrange("p (s c) -> p s c", c=C),
                                            axis=AX.X, op=ALU.add)
                    rcp = workp.tile([128, CH], f32, tag="rcp", name="rcp")
                    nc.vector.reciprocal(rcp[:, 0:cn], sm[:, 0:cn])
                    rm = workp.tile([128, CH], f32, tag="rm", name="rm")
                    nc.vector.tensor_tensor(rm[:, 0:cn], rcp[:, 0:cn],
                                            maskm[:, c0:c0 + cn], op=ALU.mult)
                    dst = workp2.tile([128, CH * C], f32, tag="dst", name="dst")
                    for s in range(cn):
                        nc.vector.tensor_scalar(dst[:, s * C:(s + 1) * C],
                                                ex[:, s * C:(s + 1) * C],
                                                rm[:, s:s + 1], None, op0=ALU.mult)
                    nc.sync.dma_start(dists_o[:, c0 * C:(c0 + cn) * C], dst[:, 0:cn * C])
                    lg_hist.clear()

                HV = [slice(0, H // 2), slice(H // 2, H)]
                for t in range(n_steps):
                    # ---- gates (tanh-only) + DVE chain, in 256-wide halves ----
                    tig = workp.tile([128, H], f32, tag="tig", name="tig")
                    tfg = workp.tile([128, H], f32, tag="tfg", name="tfg")
                    tmi = workp.tile([128, H], f32, tag="tmi", name="tmi")
                    tog = workp.tile([128, H], f32, tag="tog", name="tog")
                    thw = workp.tile([128, H], f32, tag="thw", name="thw")
                    a_ = workp.tile([128, H], f32, tag="a_", name="a_")
                    c_ = workp.tile([128, H], f32, tag="c_", name="c_")
                    m2n = workp2.tile([128, H], f32, tag="m2", name="m2")
                    tnm = workp.tile([128, H], f32, tag="tnm", name="tnm")
                    d_ = workp.tile([128, H], f32, tag="d_", name="d_")
                    f_ = workp.tile([128, H], f32, tag="f_", name="f_")
                    e_ = workp.tile([128, H], f32, tag="e_", name="e_")
                    out = workp.tile([128, H], f32, tag="out", name="out")
                    for h in HV:
                        nc.scalar.activation(tmi[:, h], pre_tiles[2][:, h], AF.Tanh)
                        nc.scalar.activation(tig[:, h], pre_tiles[0][:, h], AF.Tanh, scale=0.5)
                        nc.vector.scalar_tensor_tensor(a_[:, h], tig[:, h], 1.0, tmi[:, h],
                                                       op0=ALU.add, op1=ALU.mult)
                        nc.scalar.activation(tfg[:, h], pre_tiles[1][:, h], AF.Tanh, scale=0.5)
                        nc.vector.scalar_tensor_tensor(c_[:, h], tfg[:, h], 1.0, m2[:, h],
                                                       op0=ALU.add, op1=ALU.mult)
                        nc.vector.scalar_tensor_tensor(m2n[:, h], c_[:, h], 0.5, a_[:, h],
                                                       op0=ALU.mult, op1=ALU.add)
                        nc.scalar.activation(tnm[:, h], m2n[:, h], AF.Tanh, scale=0.5)
                        nc.scalar.activation(tog[:, h], pre_tiles[3][:, h], AF.Tanh, scale=0.5)
                        nc.vector.scalar_tensor_tensor(d_[:, h], tog[:, h], 1.0, tnm[:, h],
                                                       op0=ALU.add, op1=ALU.mult)
                        nc.scalar.activation(thw[:, h], pre_tiles[4][:, h], AF.Tanh, scale=0.5)
                        # f = (thw - 1) * p5h ; p5h = 0.5*pi5 lives in pre tile 5
                        nc.vector.scalar_tensor_tensor(f_[:, h], thw[:, h], 1.0,
                                                       pre_tiles[5][:, h],
                                                       op0=ALU.subtract, op1=ALU.mult)
                        nc.vector.scalar_tensor_tensor(e_[:, h], thw[:, h], 1.0, d_[:, h],
                                                       op0=ALU.add, op1=ALU.mult)
                        nc.vector.scalar_tensor_tensor(out[:, h], e_[:, h], 0.25, f_[:, h],
                                                       op0=ALU.mult, op1=ALU.subtract)
                    m2 = m2n

                    # ---- transpose out, split into f32r q/r (per 128-col chunk
                    #      so the first ps matmul can start early) ----
                    outT_ps = tr_ps.tile([128, 512], f32, tag="tr", name="outT_ps")
                    outq = workp.tile([128, 4, 128], f32r, tag="outq", name="outq")
                    outr = workp.tile([128, 4, 128], f32r, tag="outr", name="outr")
                    for k in range(4):
                        ck = slice(k * 128, (k + 1) * 128)
                        nc.tensor.transpose(outT_ps[:, ck], out[:, ck], ident)
                        nc.scalar.copy(outq[:, k, :], outT_ps[:, ck])
                        nc.vector.tensor_tensor(outr[:, k, :], outT_ps[:, ck],
                                                outq[:, k, :].bitcast(f32),
                                                op=ALU.subtract)

                    # ---- logits = out @ W_out.T + b_out (3-term split + bias) ----
                    lg_ps = tr_ps.tile([128, 512], f32, tag="tr", name="lg_ps")
                    lgp = lg_ps[:, 0:NOUT]
                    for k in range(4):
                        nc.tensor.matmul(lgp, outq[:, k, :], woq[:, k, :],
                                         start=(k == 0), stop=False)
                    for k in range(4):
                        nc.tensor.matmul(lgp, outq[:, k, :], wor[:, k, :], start=False, stop=False)
                    for k in range(4):
                        nc.tensor.matmul(lgp, outr[:, k, :], woq[:, k, :], start=False, stop=False)
                    nc.tensor.matmul(lgp, ones2_r, bo2, start=False, stop=True)
                    lg_sb = workp.tile([128, C], f32, tag="lg_sb", name="lg_sb",
                                       bufs=CH + 2)
                    nc.scalar.copy(lg_sb, lg_ps[:, 0:C])
                    lg_hist.append((t, lg_sb))

                    # ---- argmax over classes 1..150 (straight from PSUM,
                    #      in parallel with the SBUF copy) ----
                    mx8 = workp.tile([128, 8], f32, tag="mx8", name="mx8")
                    mi8 = workp.tile([128, 8], u32, tag="mi8", name="mi8")
                    nc.vector.max(mx8, lg_ps[:, 1:C])
                    nc.vector.max_index(mi8, mx8, lg_sb[:, 1:C])
                    nc.vector.tensor_copy(best_buf[:, t:t + 1], mi8[:, 0:1])

                    if t == n_steps - 1:
                        # masked states output (low priority, off critical path)
                        sout = workp.tile([128, H], f32, tag="sout", name="sout")
                        nc.scalar.mul(sout, out, maskm[:, t:t + 1])
                        nc.sync.dma_start(states_o[:, t, :], sout)
                        softmax_chunk()
                        break

                    # ---- onehot (full row vs max of [1:]) -> transposed f32r ----
                    nc.vector.tensor_scalar(oh_bm, lg_ps[:, 0:128], mx8[:, 0:1], None,
                                            op0=ALU.is_equal)
                    nc.vector.tensor_scalar(oh2[:, 0:C - 128], lg_ps[:, 128:C],
                                            mx8[:, 0:1], None, op0=ALU.is_equal)
                    nc.vector.tensor_scalar(oh2[:, 32:32 + C - 128], lg_ps[:, 128:C],
                                            mx8[:, 0:1], None, op0=ALU.is_equal)
                    ohT_ps = tr_ps.tile([128, 512], f32, tag="tr", name="ohT_ps")
                    nc.tensor.transpose(ohT_ps[:, 0:128], oh_bm, ident)
                    nc.tensor.transpose(ohT_ps[0:64, 128:256], oh2, ident)
                    nc.scalar.copy(ohT0, ohT_ps[:, 0:128])
                    nc.scalar.copy(ohT1, ohT_ps[0:64, 128:256])

                    # ---- preact_{t+1} = ps (state, only needs the split) then
                    #      pi (onehot, needs the argmax chain) accumulated on top.
                    #      ps first keeps the PE busy during argmax/onehot.
                    new_tiles = [None] * NTILE
                    for j in (2, 0, 1, 3, 4):
                        nj = slice(j * 512, (j + 1) * 512)
                        pt = pre_tile()
                        new_tiles[j] = pt
                        for term, (lh, rh) in enumerate(
                                ((outq, wsq), (outq, wsr), (outr, wsq))):
                            for k in range(4):
                                nc.tensor.matmul(pt, lh[:, k, :], rh[:, k, nj],
                                                 start=(term == 0 and k == 0),
                                                 stop=False)
                    new_tiles[5] = pre_tile()
                    for j in (2, 0, 1, 3, 4, 5):
                        nj = slice(j * 512, (j + 1) * 512)
                        pt = new_tiles[j]
                        nc.tensor.matmul(pt, ohT0, paq0[:, nj], start=(j == 5), stop=False)
                        nc.tensor.matmul(pt, ohT1, pa1s[:, nj], start=False, stop=False)
                        nc.tensor.matmul(pt, ohT0, par0[:, nj], start=False, stop=True)
                    pre_tiles = new_tiles

                    # ---- masked states output (off critical path) ----
                    sout = workp.tile([128, H], f32, tag="sout", name="sout")
                    nc.vector.tensor_scalar(sout, out, maskm[:, t:t + 1], None,
                                            op0=ALU.mult)
                    nc.sync.dma_start(states_o[:, t, :], sout)
                    if len(lg_hist) == CH:
                        softmax_chunk()

                # ---- best indices out ----
                nc.sync.dma_start(best_o[:, 0:n_steps], best_buf[:, 0:n_steps])

    nc.compile()
    return nc


def _host_consts(inp):
    """Host-side constant preparation (weight transposes, table, splits)."""
    W_in = np.asarray(inp["W_in"], np.float32)
    b_in = np.asarray(inp["b_in"], np.float32)
    W_state = np.asarray(inp["W_state"], np.float32)
    b_state = np.asarray(inp["b_state"], np.float32)
    W_out = np.asarray(inp["W_out"], np.float32)
    b_out = np.asarray(inp["b_out"], np.float32)
    W_onehot = np.asarray(inp["W_onehot"], np.float32)
    b_onehot = np.asarray(inp["b_onehot"], np.float32)
    W_embed = np.asarray(inp["W_embed"], np.float32)
    b_embed = np.asarray(inp["b_embed"], np.float32)

    # P_aug: class rows = W_onehot[:,c] @ W_in.T ; bias row folds
    # b_onehot @ W_in.T + b_in + [b_state | 0].  Chunk-5 columns pre-halved.
    P0 = (W_onehot.T @ W_in.T).astype(np.float32)          # (C, 6H)
    brow = ((b_onehot @ W_in.T).astype(np.float32)
            + b_in + np.concatenate([b_state, np.zeros(H, np.float32)])).astype(np.float32)
    P_aug = np.concatenate([P0, brow[None, :]], axis=0)    # (152, 6H)
    P_aug[:, 5 * H:] *= np.float32(0.5)
    paq, par = split12(P_aug)

    wsT = np.ascontiguousarray(W_state.T)                  # (H, 5H)
    wsq, wsr = split12(wsT)

    woT = np.zeros((H, NOUT), np.float32)
    woT[:, :C] = W_out.T
    woq, wor = split12(woT)
    bo = np.zeros((1, NOUT), np.float32)
    bo[0, :C] = b_out
    boq, bor = split12(bo)
    bo2 = np.concatenate([boq, bor], axis=0)
    pa1s = np.zeros((64, H6), np.float32)
    pa1s[0:CA - 128] = paq[128:]
    pa1s[32:32 + CA - 128] = par[128:]

    winT = np.ascontiguousarray(W_in.T).copy()             # (H, 6H)
    winT[:, 5 * H:] *= np.float32(0.5)
    binrow = (b_in + np.concatenate([b_state, np.zeros(H, np.float32)])).astype(np.float32)
    binrow = binrow.copy()
    binrow[5 * H:] *= np.float32(0.5)

    return {
        "wsq": wsq, "wsr": wsr,
        "paq0": paq[:128], "par0": par[:128], "pa1s": pa1s,
        "woq": woq, "wor": wor, "bo2": bo2,
        "wembT": np.ascontiguousarray(W_embed.T), "bemb": b_embed[None, :].astype(np.float32),
        "winT": winT, "binrow": binrow[None, :],
        "ident": np.eye(128, dtype=np.float32),
        "onesr": np.ones((1, 128), np.float32),
        "ones2": np.ones((2, 128), np.float32),
    }


def kernel(**inputs):
    x = np.asarray(inputs["inputs"], np.float32)
    obj = np.asarray(inputs["obj_num"]).astype(np.int64)
    consts = _host_consts(inputs)

    if "nc" not in _CACHE:
        _CACHE["nc"] = build_kernel(T)
    nc = _CACHE["nc"]

    maskf = (np.arange(T)[None, :] < obj[:, None]).astype(np.float32)  # (B, T)
    in_maps = []
    for c in range(NCORES):
        sl = slice(c * BC, (c + 1) * BC)
        m = dict(consts)
        m["x"] = np.ascontiguousarray(x[sl])
        m["maskm"] = np.ascontiguousarray(maskf[sl])
        in_maps.append(m)

    res = run_bass_kernel_spmd(nc, in_maps, core_ids=list(range(NCORES)))

    dists = np.concatenate([r["dists_o"].reshape(BC, T, C) for r in res.results], axis=0)
    states = np.concatenate([r["states_o"] for r in res.results], axis=0)
    best_bt = np.concatenate([r["best_o"] for r in res.results], axis=0)  # (B, T)
    out_commitments = np.ascontiguousarray(best_bt.T).reshape(-1).astype(np.int32)
    return dists, out_commitments, states


# revision 35
# speedup vs baseline: 1.0496x; 1.0496x over previous
"""Trainium2 Bass kernel for nn_MultiLabelRNN.

Reference semantics (per batch row b, T=65 steps):
  embed0 = inputs @ W_embed.T + b_embed
  step: pi = embed @ W_in.T + b_in ; ps = state @ W_state.T + b_state
        gates (sigmoid/tanh), highway; logits = out @ W_out.T + b_out
        dist = softmax(logits); onehot = (dist == max(dist[:,1:]))
        next embed = onehot @ W_onehot.T + b_onehot
Key restructurings:
  * embed_{t+1} has only 151 possible values -> pi_{t+1} = onehotAug @ P_aug where
    P_aug = [W_onehot[:,c] @ W_in.T rows ; bias row], bias row folds
    b_onehot@W_in.T + b_in + [b_state|0].  Chunk-5 columns (the highway skip
    input pi5) are pre-halved so the DVE chain can use them directly.
  * All matmuls run as float32r (TF32-like, full PE rate at N>=256) in split
    form: x@W ~= xq@Wq + xq@Wr + xr@Wq with q/r an exact hi/lo decomposition
    (11-bit mantissa pieces; products are exact on the PE, fp32 accumulate in
    PSUM) -> fp32-grade accuracy at ~3x the speed of native fp32 matmul.
    Constant weights are split on the host; the recurrent state is split on
    device (ACT cast + DVE subtract).
  * Gates use tanh only (ACT tanh is ~4x more accurate than ACT sigmoid):
    sigmoid(x) = 0.5 + 0.5*tanh(x/2), memory is carried doubled (M2 = 2*mem).
  * argmax/onehot from logits (softmax is monotone); dists are computed in a
    decoupled end phase from logits stored to an HBM scratch buffer.
Sharding: data-parallel over batch, 128 rows per core on 8 cores.
"""

import numpy as np

import concourse.bacc as bacc
import concourse.mybir as mybir
from concourse.tile import TileContext
from concourse.bass_utils import run_bass_kernel_spmd

B, D, H, C, T = 1024, 2048, 512, 151, 65
NCORES = 8
BC = B // NCORES  # 128 batch rows per core
H6 = 6 * H
NTILE = H6 // 512  # 6 preact tiles of 512 columns
NOUT = 256  # padded logits width (f32r needs N>=256 for full rate)
CA = C + 1  # onehot rows + constant-one bias row

f32 = mybir.dt.float32
f32r = mybir.dt.float32r
i32 = mybir.dt.int32
u32 = mybir.dt.uint32
AF = mybir.ActivationFunctionType
ALU = mybir.AluOpType
AX = mybir.AxisListType

_CACHE = {}


def round12(x):
    """Round fp32 to 11 explicit mantissa bits (f32r grid), half-to-even.
    Matches the device's f32r cast bit-exactly (verified on HW)."""
    v = np.ascontiguousarray(x, dtype=np.float32).view(np.uint32)
    keep = v & np.uint32(0xFFFFF000)
    rem = v & np.uint32(0x00000FFF)
    lsb = (v >> np.uint32(12)) & np.uint32(1)
    up = (rem > 0x800) | ((rem == 0x800) & (lsb == 1))
    return (keep + (up.astype(np.uint32) << np.uint32(12))).view(np.float32)


def split12(x):
    q = round12(x)
    r = round12((x.astype(np.float32) - q).astype(np.float32))
    return q, r


def build_kernel(n_steps=T):
    nc = bacc.Bacc(target_bir_lowering=False)

    # ---- I/O ----
    x_d = nc.dram_tensor("x", [BC, D], f32, kind="ExternalInput")
    wsq_d = nc.dram_tensor("wsq", [H, 5 * H], f32r, kind="ExternalInput")
    wsr_d = nc.dram_tensor("wsr", [H, 5 * H], f32r, kind="ExternalInput")
    paq0_d = nc.dram_tensor("paq0", [128, H6], f32r, kind="ExternalInput")
    par0_d = nc.dram_tensor("par0", [128, H6], f32r, kind="ExternalInput")
    # stacked chunk-1 table: rows 0..23 = q piece, rows 32..55 = r piece,
    # zero padding rows 24..31 / 56..63 (lets one K=64 matmul do both terms)
    pa1s_d = nc.dram_tensor("pa1s", [64, H6], f32r, kind="ExternalInput")
    woq_d = nc.dram_tensor("woq", [H, NOUT], f32r, kind="ExternalInput")
    wor_d = nc.dram_tensor("wor", [H, NOUT], f32r, kind="ExternalInput")
    bo2_d = nc.dram_tensor("bo2", [2, NOUT], f32r, kind="ExternalInput")
    wembT_d = nc.dram_tensor("wembT", [D, H], f32, kind="ExternalInput")
    bemb_d = nc.dram_tensor("bemb", [1, H], f32, kind="ExternalInput")
    winT_d = nc.dram_tensor("winT", [H, H6], f32, kind="ExternalInput")
    binrow_d = nc.dram_tensor("binrow", [1, H6], f32, kind="ExternalInput")
    mask_d = nc.dram_tensor("maskm", [BC, T], f32, kind="ExternalInput")
    ident_d = nc.dram_tensor("ident", [128, 128], f32, kind="ExternalInput")
    onesr_d = nc.dram_tensor("onesr", [1, 128], f32, kind="ExternalInput")
    ones2_d = nc.dram_tensor("ones2", [2, 128], f32r, kind="ExternalInput")

    dists_o = nc.dram_tensor("dists_o", [BC, T * C], f32, kind="ExternalOutput")
    best_o = nc.dram_tensor("best_o", [BC, T], i32, kind="ExternalOutput")
    states_o = nc.dram_tensor("states_o", [BC, T, H], f32, kind="ExternalOutput")

    with TileContext(nc) as tc:
        with (
            tc.tile_pool(name="tiny", bufs=1) as tiny,
            tc.tile_pool(name="pre_ps", bufs=6, space="PSUM") as pre_ps,
            tc.tile_pool(name="tr_ps", bufs=2, space="PSUM") as tr_ps,
        ):
            ident = tiny.tile([128, 128], f32)
            onesr = tiny.tile([1, 128], f32)
            nc.sync.dma_start(ident, ident_d[:, :])
            nc.sync.dma_start(onesr, onesr_d[:, :])

            def pre_tile():
                t_ = pre_ps.tile([128, 512], f32, tag="pre", name="pre_t")
                return t_

            # ================= init phase: embed0 -> preact_0 =================
            pre_tiles = None
            with tc.tile_pool(name="init", bufs=1) as initp:
                x_bm = initp.tile([128, D], f32)
                nc.sync.dma_start(x_bm, x_d[:, :])
                # transpose inputs -> xT (D x 128) as 16 chunks
                xT = initp.tile([128, D // 128, 128], f32)
                for k in range(D // 128):
                    xt_ps = tr_ps.tile([128, 512], f32, tag="tr", name="xt_ps")
                    nc.tensor.transpose(xt_ps[:, 0:128], x_bm[:, k * 128:(k + 1) * 128], ident)
                    nc.scalar.copy(xT[:, k, :], xt_ps[:, 0:128])
                # embed0 (batch-major) = x @ W_embed.T + b_embed  (fp32 matmuls)
                bemb_sb = initp.tile([1, H], f32)
                nc.sync.dma_start(bemb_sb, bemb_d[:, :])
                e0_ps = tr_ps.tile([128, 512], f32, tag="tr", name="e0_ps")
                with tc.tile_pool(name="wemb", bufs=2) as wembp:
                    for k in range(D // 128):
                        wch = wembp.tile([128, H], f32, tag="wch", name="wch")
                        nc.sync.dma_start(wch, wembT_d[k * 128:(k + 1) * 128, :])
                        nc.tensor.matmul(e0_ps, xT[:, k, :], wch, start=(k == 0), stop=False)
                    nc.tensor.matmul(e0_ps, onesr, bemb_sb, start=False, stop=True)
                e0_bm = initp.tile([128, H], f32)
                nc.scalar.copy(e0_bm, e0_ps)
                # transpose embed0 -> e0T (4 chunks)
                e0T_ps = tr_ps.tile([128, 512], f32, tag="tr", name="e0T_ps")
                for k in range(4):
                    nc.tensor.transpose(e0T_ps[:, k * 128:(k + 1) * 128],
                                        e0_bm[:, k * 128:(k + 1) * 128], ident)
                e0T = initp.tile([128, 4, 128], f32)
                nc.scalar.copy(e0T, e0T_ps.rearrange("p (k m) -> p k m", k=4))
                # preact_0 = embed0 @ W_in.T + (b_in + b_state_pad)   (fp32)
                binrow_sb = initp.tile([1, H6], f32)
                nc.sync.dma_start(binrow_sb, binrow_d[:, :])
                pre_tiles = [pre_tile() for _ in range(NTILE)]
                with tc.tile_pool(name="winp", bufs=2) as winp:
                    for k in range(4):
                        wik = winp.tile([128, H6], f32, tag="wik", name="wik")
                        nc.sync.dma_start(wik, winT_d[k * 128:(k + 1) * 128, :])
                        for j in range(NTILE):
                            nc.tensor.matmul(pre_tiles[j], e0T[:, k, :],
                                             wik[:, j * 512:(j + 1) * 512],
                                             start=(k == 0), stop=False)
                    for j in range(NTILE):
                        nc.tensor.matmul(pre_tiles[j], onesr,
                                         binrow_sb[:, j * 512:(j + 1) * 512],
                                         start=False, stop=True)

            # ================= constants for the main loop =================
            with (
                tc.tile_pool(name="const", bufs=1) as constp,
                tc.tile_pool(name="work", bufs=1) as workp,
                tc.tile_pool(name="work2", bufs=2) as workp2,
            ):
                wsq = constp.tile([128, 4, 5 * H], f32r)
                wsr = constp.tile([128, 4, 5 * H], f32r)
                nc.sync.dma_start(wsq, wsq_d.rearrange("(k p) n -> p k n", p=128))
                nc.sync.dma_start(wsr, wsr_d.rearrange("(k p) n -> p k n", p=128))
                paq0 = constp.tile([128, H6], f32r)
                par0 = constp.tile([128, H6], f32r)
                pa1s = constp.tile([64, H6], f32r)
                nc.sync.dma_start(paq0, paq0_d[:, :])
                nc.sync.dma_start(par0, par0_d[:, :])
                nc.sync.dma_start(pa1s, pa1s_d[:, :])
                woq = constp.tile([128, 4, NOUT], f32r)
                wor = constp.tile([128, 4, NOUT], f32r)
                nc.sync.dma_start(woq, woq_d.rearrange("(k p) n -> p k n", p=128))
                nc.sync.dma_start(wor, wor_d.rearrange("(k p) n -> p k n", p=128))
                bo2 = constp.tile([2, NOUT], f32r)
                nc.sync.dma_start(bo2, bo2_d[:, :])
                ones2_r = constp.tile([2, 128], f32r)
                nc.sync.dma_start(ones2_r, ones2_d[:, :])
                maskm = constp.tile([BC, T], f32)
                nc.sync.dma_start(maskm, mask_d[:, :])
                best_buf = constp.tile([BC, T], i32)

                # onehot batch-major staging: cols 0..150 = compare output,
                # col 151 = constant 1.0 (bias-row selector).  Persistent
                # tensors (not pool-rotated) so the constant column survives.
                oh_bm = constp.tile([128, 128], f32)
                # oh2: cols 0..22 = onehot classes 128..150 (q side), col 23 = 1
                # (bias-row selector), cols 32..54 = classes again (r side),
                # col 55 = 1, rest zero.  Its transpose is the stacked chunk-1
                # lhsT for the single K=64 pi matmul.
                oh2 = constp.tile([128, 64], f32)
                nc.vector.memset(oh2, 0.0)
                nc.vector.memset(oh2[:, 23:24], 1.0)
                nc.vector.memset(oh2[:, 55:56], 1.0)
                ohT0 = constp.tile([128, 128], f32r)
                ohT1 = constp.tile([64, 128], f32r)

                # M2 (= 2*memory) ping-pong, init zero
                m2 = workp2.tile([128, H], f32, tag="m2", name="m2")
                nc.vector.memset(m2, 0.0)

                CH = 5  # inline-softmax chunk (steps)
                lg_hist = []  # (t, lg_sb tile) pending softmax

                def softmax_chunk():
                    """Emit softmax for the pending lg tiles (overlaps PE work)."""
                    cn = len(lg_hist)
                    if cn == 0:
                        return
                    c0 = lg_hist[0][0]
                    ex = workp2.tile([128, CH * C], f32, tag="ex", name="ex")
                    for s, (tt, lgt) in enumerate(lg_hist):
                        nmx = workp.tile([128, 1], f32, tag="nmx", name="nmx")
                        nc.vector.tensor_reduce(nmx, lgt, axis=AX.X, op=ALU.max,
                                                negate=True)
                        nc.scalar.activation(ex[:, s * C:(s + 1) * C], lgt, AF.Exp,
                                             bias=nmx)
                    sm = workp.tile([128, CH], f32, tag="sm", name="sm")
                    nc.vector.tensor_reduce(sm[:, 0:cn],
                                            ex[:, 0:cn * C].rearrange("p (s c) -> p s c", c=C),
                                            axis=AX.X, op=ALU.add)
                    rcp = workp.tile([128, CH], f32, tag="rcp", name="rcp")
                    nc.vector.reciprocal(rcp[:, 0:cn], sm[:, 0:cn])
                    rm = workp.tile([128, CH], f32, tag="rm", name="rm")
                    nc.vector.tensor_tensor(rm[:, 0:cn], rcp[:, 0:cn],
                                            maskm[:, c0:c0 + cn], op=ALU.mult)
                    dst = workp2.tile([128, CH * C], f32, tag="dst", name="dst")
                    for s in range(cn):
                        nc.vector.tensor_scalar(dst[:, s * C:(s + 1) * C],
                                                ex[:, s * C:(s + 1) * C],
                                                rm[:, s:s + 1], None, op0=ALU.mult)
                    nc.sync.dma_start(dists_o[:, c0 * C:(c0 + cn) * C], dst[:, 0:cn * C])
                    lg_hist.clear()

                HV = [slice(0, H // 2), slice(H // 2, H)]
                for t in range(n_steps):
                    # ---- gates (tanh-only) + DVE chain, in 256-wide halves ----
                    tig = workp.tile([128, H], f32, tag="tig", name="tig")
                    tfg = workp.tile([128, H], f32, tag="tfg", name="tfg")
                    tmi = workp.tile([128, H], f32, tag="tmi", name="tmi")
                    tog = workp.tile([128, H], f32, tag="tog", name="tog")
                    thw = workp.tile([128, H], f32, tag="thw", name="thw")
                    a_ = workp.tile([128, H], f32, tag="a_", name="a_")
                    c_ = workp.tile([128, H], f32, tag="c_", name="c_")
                    m2n = workp2.tile([128, H], f32, tag="m2", name="m2")
                    tnm = workp.tile([128, H], f32, tag="tnm", name="tnm")
                    d_ = workp.tile([128, H], f32, tag="d_", name="d_")
                    f_ = workp.tile([128, H], f32, tag="f_", name="f_")
                    e_ = workp.tile([128, H], f32, tag="e_", name="e_")
                    out = workp.tile([128, H], f32, tag="out", name="out")
                    for h in HV:
                        nc.scalar.activation(tmi[:, h], pre_tiles[2][:, h], AF.Tanh)
                        nc.scalar.activation(tig[:, h], pre_tiles[0][:, h], AF.Tanh, scale=0.5)
                        nc.vector.scalar_tensor_tensor(a_[:, h], tig[:, h], 1.0, tmi[:, h],
                                                       op0=ALU.add, op1=ALU.mult)
                        nc.scalar.activation(tfg[:, h], pre_tiles[1][:, h], AF.Tanh, scale=0.5)
                        nc.vector.scalar_tensor_tensor(c_[:, h], tfg[:, h], 1.0, m2[:, h],
                                                       op0=ALU.add, op1=ALU.mult)
                        nc.vector.scalar_tensor_tensor(m2n[:, h], c_[:, h], 0.5, a_[:, h],
                                                       op0=ALU.mult, op1=ALU.add)
                        nc.scalar.activation(tnm[:, h], m2n[:, h], AF.Tanh, scale=0.5)
                        nc.scalar.activation(tog[:, h], pre_tiles[3][:, h], AF.Tanh, scale=0.5)
                        nc.vector.scalar_tensor_tensor(d_[:, h], tog[:, h], 1.0, tnm[:, h],
                                                       op0=ALU.add, op1=ALU.mult)
                        nc.scalar.activation(thw[:, h], pre_tiles[4][:, h], AF.Tanh, scale=0.5)
                        # f = (thw - 1) * p5h ; p5h = 0.5*pi5 lives in pre tile 5
                        nc.vector.scalar_tensor_tensor(f_[:, h], thw[:, h], 1.0,
                                                       pre_tiles[5][:, h],
                                                       op0=ALU.subtract, op1=ALU.mult)
                        nc.vector.scalar_tensor_tensor(e_[:, h], thw[:, h], 1.0, d_[:, h],
                                                       op0=ALU.add, op1=ALU.mult)
                        nc.vector.scalar_tensor_tensor(out[:, h], e_[:, h], 0.25, f_[:, h],
                                                       op0=ALU.mult, op1=ALU.subtract)
                    m2 = m2n

                    # ---- transpose out, split into f32r q/r (per 128-col chunk
                    #      so the first ps matmul can start early) ----
                    outT_ps = tr_ps.tile([128, 512], f32, tag="tr", name="outT_ps")
                    outq = workp.tile([128, 4, 128], f32r, tag="outq", name="outq")
                    outr = workp.tile([128, 4, 128], f32r, tag="outr", name="outr")
                    for k in range(4):
                        ck = slice(k * 128, (k + 1) * 128)
                        nc.tensor.transpose(outT_ps[:, ck], out[:, ck], ident)
                        nc.scalar.copy(outq[:, k, :], outT_ps[:, ck])
                        nc.vector.tensor_tensor(outr[:, k, :], outT_ps[:, ck],
                                                outq[:, k, :].bitcast(f32),
                                                op=ALU.subtract)

                    # ---- logits = out @ W_out.T + b_out (3-term split + bias) ----
                    lg_ps = tr_ps.tile([128, 512], f32, tag="tr", name="lg_ps")
                    lgp = lg_ps[:, 0:NOUT]
                    for k in range(4):
                        nc.tensor.matmul(lgp, outq[:, k, :], woq[:, k, :],
                                         start=(k == 0), stop=False)
                    for k in range(4):
                        nc.tensor.matmul(lgp, outq[:, k, :], wor[:, k, :], start=False, stop=False)
                    for k in range(4):
                        nc.tensor.matmul(lgp, outr[:, k, :], woq[:, k, :], start=False, stop=False)
                    nc.tensor.matmul(lgp, ones2_r, bo2, start=False, stop=True)
                    lg_sb = workp.tile([128, C], f32, tag="lg_sb", name="lg_sb",
                                       bufs=CH + 2)
                    nc.scalar.copy(lg_sb, lg_ps[:, 0:C])
                    lg_hist.append((t, lg_sb))

                    # ---- argmax over classes 1..150 (straight from PSUM,
                    #      in parallel with the SBUF copy) ----
                    mx8 = workp.tile([128, 8], f32, tag="mx8", name="mx8")
                    mi8 = workp.tile([128, 8], u32, tag="mi8", name="mi8")
                    nc.vector.max(mx8, lg_ps[:, 1:C])
                    nc.vector.max_index(mi8, mx8, lg_sb[:, 1:C])
                    nc.vector.tensor_copy(best_buf[:, t:t + 1], mi8[:, 0:1])

                    if t == n_steps - 1:
                        # masked states output (low priority, off critical path)
                        sout = workp.tile([128, H], f32, tag="sout", name="sout")
                        nc.scalar.mul(sout, out, maskm[:, t:t + 1])
                        nc.sync.dma_start(states_o[:, t, :], sout)
                        softmax_chunk()
                        break

                    # ---- onehot (full row vs max of [1:]) -> transposed f32r ----
                    nc.vector.tensor_scalar(oh_bm, lg_ps[:, 0:128], mx8[:, 0:1], None,
                                            op0=ALU.is_equal)
                    nc.vector.tensor_scalar(oh2[:, 0:C - 128], lg_ps[:, 128:C],
                                            mx8[:, 0:1], None, op0=ALU.is_equal)
                    nc.vector.tensor_scalar(oh2[:, 32:32 + C - 128], lg_ps[:, 128:C],
                                            mx8[:, 0:1], None, op0=ALU.is_equal)
                    ohT_ps = tr_ps.tile([128, 512], f32, tag="tr", name="ohT_ps")
                    nc.tensor.transpose(ohT_ps[:, 0:128], oh_bm, ident)
                    nc.tensor.transpose(ohT_ps[0:64, 128:256], oh2, ident)
                    nc.scalar.copy(ohT0, ohT_ps[:, 0:128])
                    nc.scalar.copy(ohT1, ohT_ps[0:64, 128:256])

                    # ---- preact_{t+1} = ps (state, only needs the split) then
                    #      pi (onehot, needs the argmax chain) accumulated on top.
                    #      ps first keeps the PE busy during argmax/onehot.
                    new_tiles = [None] * NTILE

                    def emit_ps(j):
                        nj = slice(j * 512, (j + 1) * 512)
                        pt = pre_tile()
                        new_tiles[j] = pt
                        for term, (lh, rh) in enumerate(
                                ((outq, wsq), (outq, wsr), (outr, wsq))):
                            for k in range(4):
                                nc.tensor.matmul(pt, lh[:, k, :], rh[:, k, nj],
                                                 start=(term == 0 and k == 0),
                                                 stop=False)

                    def emit_pi(j, start=False):
                        nj = slice(j * 512, (j + 1) * 512)
                        pt = new_tiles[j]
                        nc.tensor.matmul(pt, ohT0, paq0[:, nj], start=start, stop=False)
                        nc.tensor.matmul(pt, ohT1, pa1s[:, nj], start=False, stop=False)
                        nc.tensor.matmul(pt, ohT0, par0[:, nj], start=False, stop=True)

                    # Interleave so early tiles complete (stop) quickly and the
                    # next step's gates overlap the tail of this matmul phase;
                    # the one-tile ps prefix keeps the PE fed while the
                    # argmax -> onehot chain finishes.
                    emit_ps(2)
                    emit_pi(2)
                    emit_ps(0)
                    emit_pi(0)
                    emit_ps(1)
                    emit_pi(1)
                    emit_ps(3)
                    emit_pi(3)
                    emit_ps(4)
                    emit_pi(4)
                    new_tiles[5] = pre_tile()
                    emit_pi(5, start=True)
                    pre_tiles = new_tiles

                    # ---- masked states output (off critical path) ----
                    sout = workp.tile([128, H], f32, tag="sout", name="sout")
                    nc.vector.tensor_scalar(sout, out, maskm[:, t:t + 1], None,
                                            op0=ALU.mult)
                    nc.sync.dma_start(states_o[:, t, :], sout)
                    if len(lg_hist) == CH:
                        softmax_chunk()

                # ---- best indices out ----
                nc.sync.dma_start(best_o[:, 0:n_steps], best_buf[:, 0:n_steps])

    nc.compile()
    return nc


def _host_consts(inp):
    """Host-side constant preparation (weight transposes, table, splits)."""
    W_in = np.asarray(inp["W_in"], np.float32)
    b_in = np.asarray(inp["b_in"], np.float32)
    W_state = np.asarray(inp["W_state"], np.float32)
    b_state = np.asarray(inp["b_state"], np.float32)
    W_out = np.asarray(inp["W_out"], np.float32)
    b_out = np.asarray(inp["b_out"], np.float32)
    W_onehot = np.asarray(inp["W_onehot"], np.float32)
    b_onehot = np.asarray(inp["b_onehot"], np.float32)
    W_embed = np.asarray(inp["W_embed"], np.float32)
    b_embed = np.asarray(inp["b_embed"], np.float32)

    # P_aug: class rows = W_onehot[:,c] @ W_in.T ; bias row folds
    # b_onehot @ W_in.T + b_in + [b_state | 0].  Chunk-5 columns pre-halved.
    P0 = (W_onehot.T @ W_in.T).astype(np.float32)          # (C, 6H)
    brow = ((b_onehot @ W_in.T).astype(np.float32)
            + b_in + np.concatenate([b_state, np.zeros(H, np.float32)])).astype(np.float32)
    P_aug = np.concatenate([P0, brow[None, :]], axis=0)    # (152, 6H)
    P_aug[:, 5 * H:] *= np.float32(0.5)
    paq, par = split12(P_aug)

    wsT = np.ascontiguousarray(W_state.T)                  # (H, 5H)
    wsq, wsr = split12(wsT)

    woT = np.zeros((H, NOUT), np.float32)
    woT[:, :C] = W_out.T
    woq, wor = split12(woT)
    bo = np.zeros((1, NOUT), np.float32)
    bo[0, :C] = b_out
    boq, bor = split12(bo)
    bo2 = np.concatenate([boq, bor], axis=0)
    pa1s = np.zeros((64, H6), np.float32)
    pa1s[0:CA - 128] = paq[128:]
    pa1s[32:32 + CA - 128] = par[128:]

    winT = np.ascontiguousarray(W_in.T).copy()             # (H, 6H)
    winT[:, 5 * H:] *= np.float32(0.5)
    binrow = (b_in + np.concatenate([b_state, np.zeros(H, np.float32)])).astype(np.float32)
    binrow = binrow.copy()
    binrow[5 * H:] *= np.float32(0.5)

    return {
        "wsq": wsq, "wsr": wsr,
        "paq0": paq[:128], "par0": par[:128], "pa1s": pa1s,
        "woq": woq, "wor": wor, "bo2": bo2,
        "wembT": np.ascontiguousarray(W_embed.T), "bemb": b_embed[None, :].astype(np.float32),
        "winT": winT, "binrow": binrow[None, :],
        "ident": np.eye(128, dtype=np.float32),
        "onesr": np.ones((1, 128), np.float32),
        "ones2": np.ones((2, 128), np.float32),
    }


def kernel(**inputs):
    x = np.asarray(inputs["inputs"], np.float32)
    obj = np.asarray(inputs["obj_num"]).astype(np.int64)
    consts = _host_consts(inputs)

    if "nc" not in _CACHE:
        _CACHE["nc"] = build_kernel(T)
    nc = _CACHE["nc"]

    maskf = (np.arange(T)[None, :] < obj[:, None]).astype(np.float32)  # (B, T)
    in_maps = []
    for c in range(NCORES):
        sl = slice(c * BC, (c + 1) * BC)
        m = dict(consts)
        m["x"] = np.ascontiguousarray(x[sl])
        m["maskm"] = np.ascontiguousarray(maskf[sl])
        in_maps.append(m)

    res = run_bass_kernel_spmd(nc, in_maps, core_ids=list(range(NCORES)))

    dists = np.concatenate([r["dists_o"].reshape(BC, T, C) for r in res.results], axis=0)
    states = np.concatenate([r["states_o"] for r in res.results], axis=0)
    best_bt = np.concatenate([r["best_o"] for r in res.results], axis=0)  # (B, T)
    out_commitments = np.ascontiguousarray(best_bt.T).reshape(-1).astype(np.int32)
    return dists, out_commitments, states
